# revision 27
# baseline (speedup 1.0000x reference)
"""Trainium2 Bass kernel for the additive-attention problem.

reference math:
    rec[b,h]    = sum_r rnn_state[b,r] * W_rec[h,r]
    scores[t,b] = sum_h tanh(enc[t,b,h] + rec[b,h]) * w_score[h] + b_score + mask[t,b]
    out         = softmax(scores, axis=t)          # (T, B) float32

Sharding: data-parallel over B across 8 cores (BL=4 batch columns per core).
Softmax is over T (core-local) -> no collectives.

Design (v2, h-major):
  Host prep: enc is shipped PRE-TRANSPOSED per core as [BL, H, T] fp16
  (the f32->fp16 cast forces a host copy anyway, so the transpose is free);
  rec = rnn @ W_rec.T is precomputed on host (0.03% of FLOPs) and shipped
  as a [128, HC, BL] f32 column table; w_score as [128, HC] fp16; mask
  pre-arranged to the on-chip score layout [p=t%128, (tc b)].

  Device, per chunk (b, hc) of 16 chunks  (h = hc*128 + p on partitions,
  t contiguous in free dim):
    - one contiguous 1MB DMA  enc_t[b, hc*128:...,:] -> X [128, 4096] fp16
      (first/last chunks split in t to shorten pipeline ramp and drain)
    - ONE ScalarE op: X = tanh(X + bias) with bias = rec[:, hc, b]
      (per-partition bias rides the ACT affine stage for free)
    - 32x TensorE (LDWEIGHTS[X 128x128 fp16, FWL] + MATMUL rhs=w4[:,hc])
      -> per-hc psum partial scores_ps[hc][:, tc, b].  Every matmul is a
      complete start+stop group: start=True clears has_written for the
      whole psum bank, so interleaved accumulation groups would corrupt
      neighbouring columns; the hc-sum is done on VectorE instead,
      incrementally per chunk (E[:,:,b] += partial + mask), off the
      end-of-kernel critical path.
  Tail: exp (ScalarE), PE transpose to [p=(tc,b), t%128], row sums,
  block-diag m4 matmul broadcasts per-b totals, reciprocal, scale,
  two half-DMAs out (both HWDGE rings) as (tc b) x 128 contiguous rows.

  Measured (core 0): exec ~79.5us @ nominal clock (~93.8us when the
  chip downclocks ScalarE 1.2->1.0GHz).  ACT ACTIVATE ~59.4us is the
  bottleneck (tanh is 1 elem/cycle/lane, dtype-independent; 65536
  el/partition is fixed by the problem).  DMA ~50us, PE ~34us (FWL
  works: 27ns LDWEIGHTS median), V ~7us.  Startup: ~7.1us fixed NEFF
  preamble; ACT table set preloaded via a const-AP warmup so the 1.3us
  load overlaps the first DMA; rec/w4 on the Scalar HWDGE ring.
  b_score cancels in softmax; no max-subtraction needed (|scores| < ~25).
  fp16 (not bf16) keeps the quantization error ~5e-4 (gate 2e-2).
  Rejected variants (measured): VectorE Pade(5,4) tanh offload of one
  chunk (DVE fp16 reciprocal is ~6 cyc/elem -> 102us); f32 enc with
  V-add of rec (DMA-bound 94us floor, was the 159us baseline's regime).
"""

import numpy as np

T, B, H, R = 4096, 32, 512, 512
NCORES = 8
BL = B // NCORES          # 4 local batch columns
HC = H // 128             # 4 h-chunks
TC = T // 128             # 32 t-chunks
NCOL = TC * BL            # 128 score columns (tc, b)

_GRAPH = None


def _build_graph():
    import concourse.bass as bass
    import concourse.tile as tile
    from concourse import bacc, mybir

    f32 = mybir.dt.float32
    f16 = mybir.dt.float16
    nc = bacc.Bacc()

    enc_t = nc.declare_dram_parameter("enc_t", [BL, H, T], f16, isOutput=False)
    rec_d = nc.declare_dram_parameter("rec_d", [128, HC * BL], f32, isOutput=False)
    w4d = nc.declare_dram_parameter("w4d", [128, HC], f16, isOutput=False)
    maskd = nc.declare_dram_parameter("maskd", [128, TC * BL], f32, isOutput=False)
    m4d = nc.declare_dram_parameter("m4", [128, 128], f32, isOutput=False)
    identd = nc.declare_dram_parameter("identd", [128, 128], f32, isOutput=False)
    out = nc.declare_dram_parameter("out", [BL, T], f32, isOutput=True)

    with tile.TileContext(nc) as tc_:
        with (
            tc_.tile_pool(name="singles", bufs=1) as singles,
            tc_.tile_pool(name="xpool", bufs=5) as xpool,
            tc_.tile_pool(name="spsum", bufs=1, space="PSUM") as spsum,
            tc_.tile_pool(name="tpsum", bufs=2, space="PSUM") as tpsum,
        ):
            # ---------- constants / setup ----------
            # rec/w4 gate the first tanh/matmul: issue on the Scalar
            # HWDGE ring (qActDynamicHW), which runs concurrently with
            # the Sync ring carrying the enc chunk stream
            rec_sb = singles.tile([128, HC * BL], f32)
            nc.scalar.dma_start(out=rec_sb[:], in_=rec_d[:])
            w4 = singles.tile([128, HC], f16)
            nc.scalar.dma_start(out=w4[:], in_=w4d[:])
            # preload the exp/tanh ACT table set now (the warmup input is
            # a const AP ready at preamble end, so the ~1.3us table load
            # overlaps the first enc DMA instead of serializing in front
            # of the first real tanh)
            warm = singles.tile([128, 1], f32)
            nc.scalar.activation(
                out=warm[:],
                in_=nc.const_aps.tensor(0.0, (128, 1), f32),
                func=mybir.ActivationFunctionType.Tanh,
            )
            # tail-only constants on the SWDGE ring
            ident = singles.tile([128, 128], f32)
            nc.gpsimd.dma_start(out=ident[:], in_=identd[:])
            mask_sb = singles.tile([128, TC, BL], f32)
            nc.gpsimd.dma_start(
                out=mask_sb[:], in_=maskd.rearrange("p (tc b) -> p tc b", b=BL)
            )
            m4 = singles.tile([128, 128], f32)
            nc.gpsimd.dma_start(out=m4[:], in_=m4d[:])

            # per-hc PSUM partial-score tiles: [p=t%128, (tc, b)].  Each
            # matmul is a complete single accumulation group (start+stop)
            # because start=True clears has_written bits for its whole
            # bank -- interleaved multi-matmul groups would corrupt
            # neighbouring columns.
            scores_ps = [
                spsum.tile([128, TC, BL], f32, name=f"scores{i}", tag=f"scores{i}")
                for i in range(HC)
            ]
            E = singles.tile([128, TC, BL], f32)

            # ---------- main loop over (b, hc) chunks ----------
            # The first chunks are split in t so the ScalarE tanh stream
            # starts as soon as the first 256KB lands; the last chunk is
            # split to shrink the end-of-kernel drain.
            for ci, (b, hc) in enumerate([(b, hc) for b in range(BL) for hc in range(HC)]):
                if ci == 0:
                    subs = [512, 1024, 1024, 1536]
                elif ci in (1, 2):
                    subs = [2048, 2048]
                elif ci == BL * HC - 1:
                    subs = [3584, 512]
                else:
                    subs = [T]
                X = xpool.tile([128, T], f16)
                t0 = 0
                for tsub in subs:
                    nc.sync.dma_start(
                        out=X[:, t0:t0 + tsub],
                        in_=enc_t[b, hc * 128:(hc + 1) * 128, t0:t0 + tsub],
                    )
                    # tanh(X + rec[:, hc, b]) -- one fused ScalarE pass
                    nc.scalar.activation(
                        out=X[:, t0:t0 + tsub],
                        in_=X[:, t0:t0 + tsub],
                        func=mybir.ActivationFunctionType.Tanh,
                        bias=rec_sb[:, hc * BL + b:hc * BL + b + 1],
                    )
                    for tc in range(t0 // 128, (t0 + tsub) // 128):
                        nc.tensor.matmul(
                            scores_ps[hc][:, tc, b:b + 1],
                            lhsT=X[:, tc * 128:(tc + 1) * 128],
                            rhs=w4[:, hc:hc + 1],
                            start=True,
                            stop=True,
                        )
                    t0 += tsub
                # fold this chunk's partial scores (+mask) into E as soon
                # as its matmuls land, so only one VectorE add remains
                # after the very last chunk
                if hc == 0:
                    nc.vector.tensor_add(
                        out=E[:, :, b], in0=scores_ps[0][:, :, b],
                        in1=mask_sb[:, :, b],
                    )
                else:
                    nc.vector.tensor_add(
                        out=E[:, :, b], in0=scores_ps[hc][:, :, b],
                        in1=E[:, :, b],
                    )

            # ---------- exp, softmax normalization, output ----------
            nc.scalar.activation(
                out=E[:], in_=E[:], func=mybir.ActivationFunctionType.Exp
            )
            # transpose: (p=t%128, f=(tc,b)) -> (p=(tc,b), f=t%128)
            attT = tpsum.tile([128, 128], f32)
            nc.tensor.transpose(out=attT[:], in_=E[:], identity=ident[:])
            row_sums = singles.tile([128, 1], f32)
            nc.vector.tensor_reduce(
                out=row_sums[:], in_=attT[:], axis=mybir.AxisListType.X,
                op=mybir.AluOpType.add,
            )
            # denom[p=(tc,b)] = sum over all (tc',b'==b) row sums
            denom = tpsum.tile([128, 1], f32)
            nc.tensor.matmul(
                denom[:], lhsT=m4[:], rhs=row_sums[:], start=True, stop=True
            )
            recip = singles.tile([128, 1], f32)
            nc.vector.reciprocal(out=recip[:], in_=denom[:])
            att_out = singles.tile([128, 128], f32)
            nc.vector.tensor_scalar_mul(
                out=att_out[:], in0=attT[:], scalar1=recip[:]
            )
            # partition p = (tc, b) holds 128 contiguous t values for col b.
            # Two half-DMAs on the two HWDGE rings so their ~2us HBM
            # completion receipts overlap.
            outv = out.rearrange("b (tc tp) -> tc b tp", tp=128)
            nc.sync.dma_start(out=outv[:TC // 2], in_=att_out[0:64])
            nc.scalar.dma_start(out=outv[TC // 2:], in_=att_out[64:128])

    nc.compile()
    return nc


def _get_graph():
    global _GRAPH
    if _GRAPH is None:
        _GRAPH = _build_graph()
    return _GRAPH


def make_in_maps(enc, mask, rnn_state, W_rec, w_score):
    enc = np.asarray(enc, dtype=np.float32)
    mask = np.asarray(mask, dtype=np.float32)
    # host precompute of the tiny recurrent projection: (B,R)@(R,H)->(B,H)
    rec = rnn_state.astype(np.float32) @ W_rec.astype(np.float32).T
    # w4[p, hc] = w_score[hc*128 + p]
    w4 = np.ascontiguousarray(
        w_score.astype(np.float32).reshape(HC, 128).T.astype(np.float16)
    )
    m4 = (np.arange(128)[:, None] % BL == np.arange(128)[None, :] % BL).astype(
        np.float32
    )
    ident = np.eye(128, dtype=np.float32)
    in_maps = []
    for c in range(NCORES):
        sl = slice(c * BL, (c + 1) * BL)
        # [BL, H, T] fp16, h-major with t contiguous
        enc_c = np.ascontiguousarray(
            enc[:, sl, :].transpose(1, 2, 0).astype(np.float16)
        )
        # rec_h[p, hc*BL+b] = rec[b, hc*128+p]
        rec_h = np.ascontiguousarray(
            rec[sl].T.reshape(HC, 128, BL).transpose(1, 0, 2).reshape(128, HC * BL)
        ).astype(np.float32)
        # mask_r[p, tc*BL+b] = mask[tc*128+p, b]
        mask_r = np.ascontiguousarray(
            mask[:, sl].reshape(TC, 128, BL).transpose(1, 0, 2).reshape(128, NCOL)
        )
        in_maps.append(
            {
                "enc_t": enc_c,
                "rec_d": rec_h,
                "w4d": w4,
                "maskd": mask_r,
                "m4": m4,
                "identd": ident,
            }
        )
    return in_maps


def kernel(
    encoded_contribution,
    mask,
    rnn_state,
    prev_att_weights,
    W_rec,
    w_score,
    b_score,
):
    from concourse.bass_utils import run_bass_kernel_spmd

    nc = _get_graph()
    in_maps = make_in_maps(
        np.asarray(encoded_contribution),
        np.asarray(mask),
        np.asarray(rnn_state),
        np.asarray(W_rec),
        np.asarray(w_score),
    )
    res = run_bass_kernel_spmd(nc, in_maps, list(range(NCORES)))
    outs = [np.asarray(res.results[c]["out"]) for c in range(NCORES)]
    return np.concatenate([o.T for o in outs], axis=1).astype(np.float32)
